# revision 15
# baseline (speedup 1.0000x reference)
"""Trainium2 Bass kernel for nn_Attn_6545530159401.

Computation (reference):
    enc  = encoder_outputs.transpose(1,0,2)            # (B,T,H)
    cat  = concat([hidden broadcast, enc], -1)         # (B,T,2H)
    en   = tanh(cat @ W_attn.T + b_attn)               # (B,T,H)
    sc   = en @ v                                      # (B,T)
    out  = softmax(sc, axis=1)[:, None, :]             # (B,1,T)

Split W_attn = [W_h | W_e] (each (H,H)):
    q[b]     = hidden[b] @ W_h.T + b_attn              # (B,H) tiny
    E[b,t]   = enc[b,t] @ W_e.T                        # the big matmul
    sc[b,t]  = sum_o v[o] * tanh(q[b,o] + E[b,t,o])

Sharding: data-parallel over B across 8 NeuronCores (4 batches/core),
no collectives. Per-core pipeline (o-chunks on PSUM partitions so q can
ride the ACT bias port):
    mains   PE: E-psum (128 o, 1024 rows) accumulated over 2 DoubleRow
            fp8(e4m3) matmuls (256-deep contraction each, 0.5 cy/row):
            enc quantized to e4m3 as-is, W_e scaled x128 into e4m3, the
            1/128 folded into the tanh ACT scale. Measured end-to-end
            quantization cost: rel err 1.1e-2 (< 2e-2 gate).
    tanh    ACT: tanh(E + q[b, o-chunk]) via per-partition bias
    z       DVE: z(s) = sum_o v_o * tanh_o via fused scalar_tensor_tensor
    scores  PE: sliding-window ones lhsT contracts z's 128 partitions
            into row 2s+h2 of one persistent (16, 512) psum tile
            (last superblock v-dots straight from tanh so the tail never
            waits on the DVE chain); all emissions pipelined one
            iteration behind so the in-order PE never waits on ACT/DVE
    softmax ACT exp with fused accum sum on (16, 512), per-batch
            sum/broadcast via tiny PE matmuls, no max-subtraction
            (scores are bounded by ||v||_1)
Warmup matmuls on a zeroed tile run during the first DMAs to hold the
PE clock gate (HAM) at 2.4 GHz; head DMAs are split across the two
HWDGE rings and ordered by what the PE needs first.
"""

import numpy as np
from contextlib import ExitStack

import concourse.bass as bass
import concourse.tile as tile
from concourse import bacc, mybir
import concourse.bass_utils as bass_utils

T, B, H = 2048, 32, 512
NCORES = 8
NB = B // NCORES        # 4 local batches per core
ROWS = NB * T           # 8192 rows per core
P = 128
KC = H // P             # 4 contraction chunks
OC = H // P             # 4 output chunks
SUP = 1024              # columns per E-psum tile (2 PSUM banks)
NSUP = ROWS // SUP      # 8
BLK = 512               # matmul moving-dim limit (one PSUM bank)
WSCALE = 128.0          # fp8 scale on W_e (power of 2; undone in ACT)
F32 = mybir.dt.float32
F32R = mybir.dt.float32r
F16 = mybir.dt.float16
F8 = mybir.dt.float8e4
AF = mybir.ActivationFunctionType


def _build():
    nc = bacc.Bacc(
        "TRN2", target_bir_lowering=False, debug=False, num_devices=NCORES
    )
    # enct is packed k-chunk-major: enct[p, k, r] = enc^T[k*128+p, r] (e4m3)
    enct = nc.declare_dram_parameter("enct", [P, KC, ROWS], F8, isOutput=False)
    # wetp[p, k, o] = (WSCALE*W_e^T)[k*128+p, o] (e4m3)
    wetp = nc.declare_dram_parameter("wetp", [P, KC, H], F8, isOutput=False)
    # o=0 slice of wetp: wet0[p, k, o] (o<128)
    wet0 = nc.declare_dram_parameter("wet0", [P, KC, P], F8, isOutput=False)
    whtp = nc.declare_dram_parameter("whtp", [P, KC * H], F16, isOutput=False)
    # all small constants in one (P, 64) f32 block (f16 regions bitcast):
    #   f32 cols  0:16  brep[p, o*NB+b] = b_attn[o*128+p]
    #   f32 cols 16:20  vp[p, o] = v[o*128+p]
    #   f32 cols 20:24  selb2 (rows 0:16)
    #   f32 cols 24:40  selb  (rows 0:4)
    #   f32 cols 40:56  onesw as f16[32]: ones at f16-col 15 (sliding lhsT)
    #   f32 cols 56:64  hidt as f16: chunk k at f16 cols 112+4k..116+4k
    #   f32 cols 64:128 vwin: 4 chunks of 32 f16 cols, chunk o has
    #                   v[o*128+p] at f16-col 128+32*o+15 (sliding lhsT)
    consts = nc.declare_dram_parameter("consts", [P, 128], F32, isOutput=False)
    out = nc.declare_dram_parameter("out", [NB, T], F32, isOutput=True)

    with tile.TileContext(nc) as tc, ExitStack() as ctx:
        const_pool = ctx.enter_context(tc.tile_pool(name="const", bufs=1))
        enc_pool = ctx.enter_context(tc.tile_pool(name="enc", bufs=1))
        tanh_pool = ctx.enter_context(tc.tile_pool(name="tanh", bufs=6))
        sm_pool = ctx.enter_context(tc.tile_pool(name="sm", bufs=1))
        psE_pool = ctx.enter_context(tc.tile_pool(name="psE", bufs=3, space="PSUM"))
        psS_pool = ctx.enter_context(tc.tile_pool(name="psS", bufs=1, space="PSUM"))

        # ALL DMAs stay off the Scalar ring: its sequencer time is tanh
        # time (ACT is the wall at ~43us busy; 8 DMA_DIRECT2D issues cost
        # ~7.5us of it). o=0 weight slice lands first (small, gpsimd ring)
        # in parallel with the k-chunks of the first enc superblock (sync
        # ring), so the first main matmuls can start as early as possible.
        wet0_sb = const_pool.tile([P, KC, P], F8, tag="wet0")
        nc.gpsimd.dma_start(wet0_sb[:], wet0[:, :, :])

        # whtp + consts land right after wet0 (before wetp): the o=0 slice
        # of q rides directly behind the s0/o0 main matmuls, so the first
        # tanh is not held up by the full q computation.
        whtp_sb = const_pool.tile([P, KC * H], F16, tag="whtp")
        nc.gpsimd.dma_start(whtp_sb[:], whtp[:, :])
        wht_sb = [whtp_sb[:, k * H : (k + 1) * H] for k in range(KC)]

        c_sb = const_pool.tile([P, 128], F32, tag="consts")
        nc.gpsimd.dma_start(c_sb[:], consts[:, :])

        def load_sup(s, engine=None):
            e = enc_pool.tile([P, KC, SUP], F8, tag=f"e{s}", name=f"e{s}")
            (engine or nc.sync).dma_start(
                e[:], enct[:, :, s * SUP : (s + 1) * SUP]
            )
            return e

        # whole enc shard stays SBUF-resident (4 * 32KB/partition = 128KB);
        # one 0.5MB DMA per superblock (the first one split per k-chunk so
        # matmul k=0 starts after 128KB, not 512KB)
        enc_sb = [None] * NSUP
        e0 = enc_pool.tile([P, KC, SUP], F8, tag="e0", name="e0")
        for k in range(KC):
            nc.sync.dma_start(
                e0[:, k : k + 1, :], enct[:, k : k + 1, 0:SUP]
            )
        enc_sb[0] = e0

        wetp_sb = const_pool.tile([P, KC, H], F8, tag="wetp")
        nc.gpsimd.dma_start(wetp_sb[:], wetp[:, :, :])

        c16 = c_sb[:].bitcast(F16)  # (P, 256) f16 view
        brep_sb = c_sb[:, 0:16]
        vp_sb = c_sb[:, 16:20]
        selb2_sb = c_sb[0:16, 20:24]
        selb_sb = c_sb[0:4, 24:40]
        onesw_sb = c16[:, 80:111]
        hid_sb = [c16[:, 112 + NB * k : 112 + NB * (k + 1)] for k in range(KC)]
        vwin_sb = [c16[:, 128 + 32 * o : 159 + 32 * o] for o in range(OC)]

        # late superblocks ride the gpsimd ring (idle all kernel),
        # halving the enc DMA tail on the sync ring
        for s in range(1, NSUP):
            enc_sb[s] = load_sup(
                s, engine=nc.gpsimd if s >= NSUP // 2 else nc.sync
            )

        # PE warmup: a few short matmuls on a zeroed scratch tile run while
        # the first DMAs are still in flight, so the HAM clock gate is
        # already released (2.4 GHz) when real matmuls start. Short moving
        # dim: at the cold 0.65 GHz clock each 256-row matmul is ~390ns,
        # and the first enc chunk lands ~2us after issue.
        warm = const_pool.tile([P, BLK], F16, tag="warm")
        nc.gpsimd.memset(warm[:], 0.0)
        psW = psS_pool.tile([P, BLK], F32, tag="t", name="psW")
        for _ in range(5):
            nc.tensor.matmul(
                psW[:, 0:256], lhsT=warm[:, 0:P], rhs=warm[:, 0:256],
                start=True, stop=True
            )

        # q[o, oc*4+b] = sum_h hidt[h,b] * wht[h,o] + b_attn[o]
        # (emitted one o-chunk at a time, each right after the s0/o main
        # matmuls, so tanh(s0,o) never waits on more q than it needs)
        q_sb = const_pool.tile([P, OC * NB], F32, tag="q")
        psq = psS_pool.tile([P, OC * NB], F32, tag="t", name="psq")

        def emit_q_chunk(o):
            for k in range(KC):
                nc.tensor.matmul(
                    psq[:, o * NB : (o + 1) * NB],
                    lhsT=wht_sb[k][:, o * P : (o + 1) * P],
                    rhs=hid_sb[k][:],
                    start=(k == 0),
                    stop=(k == KC - 1),
                    skip_group_check=True,
                )
            nc.vector.tensor_add(
                q_sb[:, o * NB : (o + 1) * NB],
                psq[:, o * NB : (o + 1) * NB],
                brep_sb[:, o * NB : (o + 1) * NB],
            )

        # single persistent score accumulator: row 2s+h2 = scores of
        # (batch s//2, t-slice (s%2)*1024 + h2*512)
        psS16 = psS_pool.tile([16, BLK], F32, tag="s16", name="psS16")

        # z(s) = sum_o v_o * tanh(E_o + q_o) is built on DVE (fused
        # scalar_tensor_tensor), then one small matmul per 512-block
        # contracts the 128 partitions into psS. The z-matmul for s is
        # emitted during s+1's main matmuls so the in-order PE never
        # waits on the DVE chain it just scheduled.
        def emit_zmm(z, s, b):
            for h2 in range(SUP // BLK):
                c = 2 * s + h2
                nc.tensor.matmul(
                    psS16[:],
                    lhsT=onesw_sb[:, 15 - c : 31 - c],
                    rhs=z[:, h2 * BLK : (h2 + 1) * BLK],
                    start=(s == 0 and h2 == 0),
                    stop=(s == NSUP - 1 and h2 == 1),
                )

        def emit_vdot(th, s, o):
            # direct PE v-dot (used for the last superblock so the tail
            # does not wait on the DVE z-chain)
            for h2 in range(SUP // BLK):
                c = 2 * s + h2
                nc.tensor.matmul(
                    psS16[:],
                    lhsT=vwin_sb[o][:, 15 - c : 31 - c],
                    rhs=th[:, h2 * BLK : (h2 + 1) * BLK],
                    start=False,
                    stop=(s == NSUP - 1 and o == OC - 1 and h2 == 1),
                )

        pending = None
        for s in range(NSUP):
            b = s // 2
            last_sup = s == NSUP - 1
            z = None
            for o in range(OC):
                psE = psE_pool.tile([P, SUP], F32, tag="E")
                for h2 in range(SUP // BLK):
                    for kp in range(KC // 2):
                        lhsT = (
                            wet0_sb[:, 2 * kp : 2 * kp + 2, :]
                            if o == 0
                            else wetp_sb[
                                :, 2 * kp : 2 * kp + 2, o * P : (o + 1) * P
                            ]
                        )
                        nc.tensor.matmul(
                            psE[:, h2 * BLK : (h2 + 1) * BLK],
                            lhsT=lhsT,
                            rhs=enc_sb[s][
                                :, 2 * kp : 2 * kp + 2, h2 * BLK : (h2 + 1) * BLK
                            ],
                            start=(kp == 0),
                            stop=(kp == KC // 2 - 1),
                            perf_mode=mybir.MatmulPerfMode.DoubleRow,
                        )
                if s == 0:
                    emit_q_chunk(o)
                th = tanh_pool.tile([P, SUP], F16, tag="tanh")
                nc.scalar.activation(
                    th[:],
                    psE[:],
                    AF.Tanh,
                    bias=q_sb[:, o * NB + b : o * NB + b + 1],
                    scale=1.0 / WSCALE,
                )
                if not last_sup:
                    if o == 0:
                        z = tanh_pool.tile([P, SUP], F16, tag="z", bufs=3)
                        nc.vector.tensor_scalar_mul(z[:], th[:], vp_sb[:, 0:1])
                    else:
                        nc.vector.scalar_tensor_tensor(
                            z[:],
                            th[:],
                            vp_sb[:, o : o + 1],
                            z[:],
                            op0=mybir.AluOpType.mult,
                            op1=mybir.AluOpType.add,
                        )
                if pending is not None and o == 1:
                    emit_zmm(*pending)
                    pending = None
                if last_sup and o > 0:
                    emit_vdot(prev_th, s, o - 1)
                prev_th = th
            if not last_sup:
                pending = (z, s, b)
        emit_vdot(prev_th, NSUP - 1, OC - 1)

        # softmax on the (16, 512) layout; scores are bounded (|s| <=
        # ||v||_1) so no max-subtraction is needed in f32
        ex16 = sm_pool.tile([16, BLK], F32, tag="ex16")
        sums16 = sm_pool.tile([16, 1], F32, tag="sums16")
        nc.scalar.activation(ex16[:], psS16[:], AF.Exp, accum_out=sums16[:])
        # per-batch sums: contract the 4 j-rows of each batch on PE
        psT = psS_pool.tile([NB, 1], F32, tag="t", name="psT")
        nc.tensor.matmul(
            psT[:], lhsT=selb2_sb[:], rhs=sums16[:], start=True, stop=True
        )
        rec4 = sm_pool.tile([NB, 1], F32, tag="rec4")
        nc.vector.reciprocal(rec4[:], psT[:])
        # broadcast 1/sum back to the 16 rows
        psB = psS_pool.tile([16, 1], F32, tag="t", name="psB")
        nc.tensor.matmul(
            psB[:], lhsT=selb_sb[:], rhs=rec4[:], start=True, stop=True
        )
        probs16 = sm_pool.tile([16, BLK], F32, tag="probs16")
        nc.vector.tensor_scalar_mul(probs16[:], ex16[:], psB[:, 0:1])
        nc.sync.dma_start(
            out[:, :].rearrange("b (j t) -> (b j) t", j=4), probs16[:]
        )

    nc.compile()
    return nc


_NC = None


def _get_nc():
    global _NC
    if _NC is None:
        _NC = _build()
    return _NC


def _shard_inputs(hidden, encoder_outputs, W_attn, b_attn, v):
    hidden = np.asarray(hidden, dtype=np.float32)
    encoder_outputs = np.asarray(encoder_outputs, dtype=np.float32)
    W_attn = np.asarray(W_attn, dtype=np.float32)
    b_attn = np.asarray(b_attn, dtype=np.float32)
    v = np.asarray(v, dtype=np.float32)

    import ml_dtypes

    F8NP = ml_dtypes.float8_e4m3

    # wetp[p, k, o] = (WSCALE*W_e^T)[k*128+p, o] (e4m3);
    # whtp[p, k*H + o] = W_h^T[k*128+p, o] (f16)
    wet_t = (W_attn[:, H:].T * np.float32(WSCALE)).astype(F8NP)  # (H, H)
    wht_t = W_attn[:, :H].T.astype(np.float16)
    wetp = np.ascontiguousarray(wet_t.reshape(KC, P, H).transpose(1, 0, 2))
    wet0 = np.ascontiguousarray(
        wet_t.reshape(KC, P, H)[:, :, :P].transpose(1, 0, 2)
    )
    whtp = np.ascontiguousarray(
        wht_t.reshape(KC, P, H).transpose(1, 0, 2).reshape(P, KC * H)
    )

    # packed constant block, f32 view (P, 128) / f16 view (P, 256)
    consts = np.zeros((P, 128), dtype=np.float32)
    c16 = consts.view(np.float16)  # (P, 256)
    consts[:, 0:16] = np.repeat(b_attn.reshape(OC, P).T, NB, axis=1)
    consts[:, 16:20] = v.reshape(OC, P).T
    for b in range(NB):
        for j in range(NB):
            consts[NB * b + j, 20 + b] = 1.0  # selb2 (rows 0:16)
            consts[b, 24 + NB * b + j] = 1.0  # selb (rows 0:4)
    c16[:, 80 + 15] = np.float16(1.0)  # onesw: ones at f16-col 15
    vrT = v.reshape(OC, P).T.astype(np.float16)  # (P, OC)
    for o in range(OC):
        c16[:, 128 + 32 * o + 15] = vrT[:, o]  # vwin sliding windows

    # (H, B, T) so per-core slices are cheap views before the copy
    enc_hbt = np.transpose(encoder_outputs, (2, 1, 0))
    in_maps = []
    for c in range(NCORES):
        b0 = c * NB
        # enct[p, k, r] = enc^T[k*128+p, r] (e4m3)
        enct = (
            np.ascontiguousarray(enc_hbt[:, b0 : b0 + NB, :])
            .astype(F8NP)
            .reshape(KC, P, ROWS)
        )
        enct = np.ascontiguousarray(enct.transpose(1, 0, 2))
        cc = consts.copy()
        cc16 = cc.view(np.float16)
        hidt = hidden[0, b0 : b0 + NB, :].T.astype(np.float16)  # (H, NB)
        for k in range(KC):
            cc16[:, 112 + NB * k : 112 + NB * (k + 1)] = hidt[
                k * P : (k + 1) * P, :
            ]
        in_maps.append(
            {
                "enct": enct,
                "wetp": wetp,
                "wet0": wet0,
                "whtp": whtp,
                "consts": cc,
            }
        )
    return in_maps


def kernel(hidden, encoder_outputs, W_attn, b_attn, v):
    nc = _get_nc()
    in_maps = _shard_inputs(hidden, encoder_outputs, W_attn, b_attn, v)
    res = bass_utils.run_bass_kernel_spmd(
        nc, in_maps, core_ids=list(range(NCORES))
    )
    outs = [res.results[c]["out"] for c in range(NCORES)]  # each (NB, T)
    full = np.concatenate(outs, axis=0)  # (B, T)
    return full[:, None, :].astype(np.float32)  # (B, 1, T)



# revision 18
# speedup vs baseline: 1.0594x; 1.0594x over previous
"""Trainium2 Bass kernel for nn_Attn_6545530159401.

Computation (reference):
    enc  = encoder_outputs.transpose(1,0,2)            # (B,T,H)
    cat  = concat([hidden broadcast, enc], -1)         # (B,T,2H)
    en   = tanh(cat @ W_attn.T + b_attn)               # (B,T,H)
    sc   = en @ v                                      # (B,T)
    out  = softmax(sc, axis=1)[:, None, :]             # (B,1,T)

Split W_attn = [W_h | W_e] (each (H,H)):
    q[b]     = hidden[b] @ W_h.T + b_attn              # (B,H) tiny
    E[b,t]   = enc[b,t] @ W_e.T                        # the big matmul
    sc[b,t]  = sum_o v[o] * tanh(q[b,o] + E[b,t,o])

Sharding: data-parallel over B across 8 NeuronCores (4 batches/core),
no collectives. Per-core pipeline (o-chunks on PSUM partitions so q can
ride the ACT bias port):
    mains   PE: E-psum (128 o, 1024 rows) accumulated over 2 DoubleRow
            fp8(e4m3) matmuls (256-deep contraction each, 0.5 cy/row):
            enc quantized to e4m3 as-is, W_e scaled x128 into e4m3, the
            1/128 folded into the tanh ACT scale. Measured end-to-end
            quantization cost: rel err 1.1e-2 (< 2e-2 gate).
    tanh    ACT: tanh(E + q[b, o-chunk]) via per-partition bias
    z       DVE: z(s) = sum_o v_o * tanh_o via fused scalar_tensor_tensor
    scores  PE: sliding-window ones lhsT contracts z's 128 partitions
            into row 2s+h2 of one persistent (16, 512) psum tile
            (last superblock v-dots straight from tanh so the tail never
            waits on the DVE chain); all emissions pipelined one
            iteration behind so the in-order PE never waits on ACT/DVE
    softmax ACT exp with fused accum sum on (16, 512), per-batch
            sum/broadcast via tiny PE matmuls, no max-subtraction
            (scores are bounded by ||v||_1)
Warmup matmuls on a zeroed tile run during the first DMAs to hold the
PE clock gate (HAM) at 2.4 GHz; head DMAs are split across the two
HWDGE rings and ordered by what the PE needs first.
"""

import numpy as np
from contextlib import ExitStack

import concourse.bass as bass
import concourse.tile as tile
from concourse import bacc, mybir
import concourse.bass_utils as bass_utils

T, B, H = 2048, 32, 512
NCORES = 8
NB = B // NCORES        # 4 local batches per core
ROWS = NB * T           # 8192 rows per core
P = 128
KC = H // P             # 4 contraction chunks
OC = H // P             # 4 output chunks
SUP = 1024              # columns per E-psum tile (2 PSUM banks)
NSUP = ROWS // SUP      # 8
BLK = 512               # matmul moving-dim limit (one PSUM bank)
WSCALE = 128.0          # fp8 scale on W_e (power of 2; undone in ACT)
F32 = mybir.dt.float32
F32R = mybir.dt.float32r
F16 = mybir.dt.float16
F8 = mybir.dt.float8e4
AF = mybir.ActivationFunctionType


def _build():
    nc = bacc.Bacc(
        "TRN2", target_bir_lowering=False, debug=False, num_devices=NCORES
    )
    # enct is packed k-chunk-major: enct[p, k, r] = enc^T[k*128+p, r] (e4m3)
    enct = nc.declare_dram_parameter("enct", [P, KC, ROWS], F8, isOutput=False)
    # wetp[p, k, o] = (WSCALE*W_e^T)[k*128+p, o] (e4m3)
    wetp = nc.declare_dram_parameter("wetp", [P, KC, H], F8, isOutput=False)
    # o=0 slice of wetp: wet0[p, k, o] (o<128)
    wet0 = nc.declare_dram_parameter("wet0", [P, KC, P], F8, isOutput=False)
    whtp = nc.declare_dram_parameter("whtp", [P, KC * H], F16, isOutput=False)
    # all small constants in one (P, 64) f32 block (f16 regions bitcast):
    #   f32 cols  0:16  brep[p, o*NB+b] = b_attn[o*128+p]
    #   f32 cols 16:20  vp[p, o] = v[o*128+p]
    #   f32 cols 20:24  selb2 (rows 0:16)
    #   f32 cols 24:40  selb  (rows 0:4)
    #   f32 cols 40:56  onesw as f16[32]: ones at f16-col 15 (sliding lhsT)
    #   f32 cols 56:64  hidt as f16: chunk k at f16 cols 112+4k..116+4k
    #   f32 cols 64:128 vwin: 4 chunks of 32 f16 cols, chunk o has
    #                   v[o*128+p] at f16-col 128+32*o+15 (sliding lhsT)
    consts = nc.declare_dram_parameter("consts", [P, 128], F32, isOutput=False)
    out = nc.declare_dram_parameter("out", [NB, T], F32, isOutput=True)

    with tile.TileContext(nc) as tc, ExitStack() as ctx:
        const_pool = ctx.enter_context(tc.tile_pool(name="const", bufs=1))
        enc_pool = ctx.enter_context(tc.tile_pool(name="enc", bufs=1))
        tanh_pool = ctx.enter_context(tc.tile_pool(name="tanh", bufs=6))
        sm_pool = ctx.enter_context(tc.tile_pool(name="sm", bufs=1))
        psE_pool = ctx.enter_context(tc.tile_pool(name="psE", bufs=3, space="PSUM"))
        psS_pool = ctx.enter_context(tc.tile_pool(name="psS", bufs=1, space="PSUM"))

        # PE warmup scratch is memset on the idle Vector engine FIRST so
        # the warm matmuls can run while every DMA is still in flight
        # (gpsimd's sequencer is busy issuing DMAs for ~6us).
        warm = const_pool.tile([P, BLK], F16, tag="warm")
        nc.vector.memset(warm[:], 0.0)

        # ALL DMAs stay off the Scalar ring: its sequencer time is tanh
        # time (ACT busy ~37us; 8 DMA_DIRECT2D issues would cost ~7.5us
        # of it). DMA transfers share bandwidth in issue order, so the
        # first enc superblock goes FIRST (k01 on sync, k23 on vector as
        # two 256KB transfers), and only then the weights (gpsimd) and
        # the remaining superblocks.
        enc_sb = [None] * NSUP
        e0 = enc_pool.tile([P, KC, SUP], F8, tag="e0", name="e0")
        nc.sync.dma_start(e0[:, 0:2, :], enct[:, 0:2, 0:SUP])
        # one early issue on the scalar ring is free: ACT is idle until
        # the first tanh (~13us), and this halves the e0 landing time
        nc.scalar.dma_start(e0[:, 2:4, :], enct[:, 2:4, 0:SUP])
        enc_sb[0] = e0

        wet0_sb = const_pool.tile([P, KC, P], F8, tag="wet0")
        nc.gpsimd.dma_start(wet0_sb[:], wet0[:, :, :])

        # whtp + consts next (before wetp): the o=0 slice of q rides
        # directly behind the s0/o0 main matmuls, so the first tanh is
        # not held up by the full q computation.
        whtp_sb = const_pool.tile([P, KC * H], F16, tag="whtp")
        nc.gpsimd.dma_start(whtp_sb[:], whtp[:, :])
        wht_sb = [whtp_sb[:, k * H : (k + 1) * H] for k in range(KC)]

        c_sb = const_pool.tile([P, 128], F32, tag="consts")
        nc.gpsimd.dma_start(c_sb[:], consts[:, :])

        def load_sup(s, engine=None):
            e = enc_pool.tile([P, KC, SUP], F8, tag=f"e{s}", name=f"e{s}")
            (engine or nc.sync).dma_start(
                e[:], enct[:, :, s * SUP : (s + 1) * SUP]
            )
            return e

        wetp_sb = const_pool.tile([P, KC, H], F8, tag="wetp")
        nc.gpsimd.dma_start(wetp_sb[:], wetp[:, :, :])

        c16 = c_sb[:].bitcast(F16)  # (P, 256) f16 view
        brep_sb = c_sb[:, 0:16]
        vp_sb = c_sb[:, 16:20]
        selb2_sb = c_sb[0:16, 20:24]
        selb_sb = c_sb[0:4, 24:40]
        onesw_sb = c16[:, 80:111]
        hid_sb = [c16[:, 112 + NB * k : 112 + NB * (k + 1)] for k in range(KC)]
        vwin_sb = [c16[:, 128 + 32 * o : 159 + 32 * o] for o in range(OC)]

        # late superblocks ride the gpsimd ring (idle all kernel),
        # halving the enc DMA tail on the sync ring
        for s in range(1, NSUP):
            enc_sb[s] = load_sup(
                s, engine=nc.gpsimd if s >= NSUP // 2 else nc.sync
            )

        # PE warmup: short matmuls on the zeroed scratch tile run while
        # the first DMAs are still in flight, so the HAM clock gate is
        # already released (2.4 GHz) when real matmuls start.
        psW = psS_pool.tile([P, BLK], F32, tag="t", name="psW")
        for _ in range(6):
            nc.tensor.matmul(
                psW[:, 0:256], lhsT=warm[:, 0:P], rhs=warm[:, 0:256],
                start=True, stop=True
            )

        # q[o, oc*4+b] = sum_h hidt[h,b] * wht[h,o] + b_attn[o]
        # (emitted one o-chunk at a time, each right after the s0/o main
        # matmuls, so tanh(s0,o) never waits on more q than it needs)
        q_sb = const_pool.tile([P, OC * NB], F32, tag="q")
        psq = psS_pool.tile([P, OC * NB], F32, tag="t", name="psq")

        def emit_q_chunk(o):
            for k in range(KC):
                nc.tensor.matmul(
                    psq[:, o * NB : (o + 1) * NB],
                    lhsT=wht_sb[k][:, o * P : (o + 1) * P],
                    rhs=hid_sb[k][:],
                    start=(k == 0),
                    stop=(k == KC - 1),
                    skip_group_check=True,
                )
            nc.vector.tensor_add(
                q_sb[:, o * NB : (o + 1) * NB],
                psq[:, o * NB : (o + 1) * NB],
                brep_sb[:, o * NB : (o + 1) * NB],
            )

        # single persistent score accumulator: row 2s+h2 = scores of
        # (batch s//2, t-slice (s%2)*1024 + h2*512)
        psS16 = psS_pool.tile([16, BLK], F32, tag="s16", name="psS16")

        # z(s) = sum_o v_o * tanh(E_o + q_o) is built on DVE (fused
        # scalar_tensor_tensor), then one small matmul per 512-block
        # contracts the 128 partitions into psS. The z-matmul for s is
        # emitted during s+1's main matmuls so the in-order PE never
        # waits on the DVE chain it just scheduled.
        def emit_zmm(z, s, b):
            for h2 in range(SUP // BLK):
                c = 2 * s + h2
                nc.tensor.matmul(
                    psS16[:],
                    lhsT=onesw_sb[:, 15 - c : 31 - c],
                    rhs=z[:, h2 * BLK : (h2 + 1) * BLK],
                    start=(s == 0 and h2 == 0),
                    stop=(s == NSUP - 1 and h2 == 1),
                )

        def emit_vdot(th, s, o):
            # direct PE v-dot (used for the last superblock so the tail
            # does not wait on the DVE z-chain)
            for h2 in range(SUP // BLK):
                c = 2 * s + h2
                nc.tensor.matmul(
                    psS16[:],
                    lhsT=vwin_sb[o][:, 15 - c : 31 - c],
                    rhs=th[:, h2 * BLK : (h2 + 1) * BLK],
                    start=False,
                    stop=(s == NSUP - 1 and o == OC - 1 and h2 == 1),
                )

        pending = None
        for s in range(NSUP):
            b = s // 2
            last_sup = s == NSUP - 1
            z = None
            for o in range(OC):
                psE = psE_pool.tile([P, SUP], F32, tag="E")
                for h2 in range(SUP // BLK):
                    for kp in range(KC // 2):
                        lhsT = (
                            wet0_sb[:, 2 * kp : 2 * kp + 2, :]
                            if o == 0
                            else wetp_sb[
                                :, 2 * kp : 2 * kp + 2, o * P : (o + 1) * P
                            ]
                        )
                        nc.tensor.matmul(
                            psE[:, h2 * BLK : (h2 + 1) * BLK],
                            lhsT=lhsT,
                            rhs=enc_sb[s][
                                :, 2 * kp : 2 * kp + 2, h2 * BLK : (h2 + 1) * BLK
                            ],
                            start=(kp == 0),
                            stop=(kp == KC // 2 - 1),
                            perf_mode=mybir.MatmulPerfMode.DoubleRow,
                        )
                if s == 0:
                    emit_q_chunk(o)
                th = tanh_pool.tile([P, SUP], F16, tag="tanh")
                nc.scalar.activation(
                    th[:],
                    psE[:],
                    AF.Tanh,
                    bias=q_sb[:, o * NB + b : o * NB + b + 1],
                    scale=1.0 / WSCALE,
                )
                if not last_sup:
                    if o == 0:
                        z = tanh_pool.tile([P, SUP], F16, tag="z", bufs=3)
                        nc.vector.tensor_scalar_mul(z[:], th[:], vp_sb[:, 0:1])
                    else:
                        nc.vector.scalar_tensor_tensor(
                            z[:],
                            th[:],
                            vp_sb[:, o : o + 1],
                            z[:],
                            op0=mybir.AluOpType.mult,
                            op1=mybir.AluOpType.add,
                        )
                if pending is not None and o == 1:
                    emit_zmm(*pending)
                    pending = None
                if last_sup and o > 0:
                    emit_vdot(prev_th, s, o - 1)
                prev_th = th
            if not last_sup:
                pending = (z, s, b)
        emit_vdot(prev_th, NSUP - 1, OC - 1)

        # softmax on the (16, 512) layout; scores are bounded (|s| <=
        # ||v||_1) so no max-subtraction is needed in f32
        ex16 = sm_pool.tile([16, BLK], F32, tag="ex16")
        sums16 = sm_pool.tile([16, 1], F32, tag="sums16")
        nc.scalar.activation(ex16[:], psS16[:], AF.Exp, accum_out=sums16[:])
        # per-batch sums: contract the 4 j-rows of each batch on PE
        psT = psS_pool.tile([NB, 1], F32, tag="t", name="psT")
        nc.tensor.matmul(
            psT[:], lhsT=selb2_sb[:], rhs=sums16[:], start=True, stop=True
        )
        rec4 = sm_pool.tile([NB, 1], F32, tag="rec4")
        nc.vector.reciprocal(rec4[:], psT[:])
        # broadcast 1/sum back to the 16 rows
        psB = psS_pool.tile([16, 1], F32, tag="t", name="psB")
        nc.tensor.matmul(
            psB[:], lhsT=selb_sb[:], rhs=rec4[:], start=True, stop=True
        )
        probs16 = sm_pool.tile([16, BLK], F32, tag="probs16")
        nc.vector.tensor_scalar_mul(probs16[:], ex16[:], psB[:, 0:1])
        nc.sync.dma_start(
            out[:, :].rearrange("b (j t) -> (b j) t", j=4), probs16[:]
        )

    nc.compile()
    return nc


_NC = None


def _get_nc():
    global _NC
    if _NC is None:
        _NC = _build()
    return _NC


def _shard_inputs(hidden, encoder_outputs, W_attn, b_attn, v):
    hidden = np.asarray(hidden, dtype=np.float32)
    encoder_outputs = np.asarray(encoder_outputs, dtype=np.float32)
    W_attn = np.asarray(W_attn, dtype=np.float32)
    b_attn = np.asarray(b_attn, dtype=np.float32)
    v = np.asarray(v, dtype=np.float32)

    import ml_dtypes

    F8NP = ml_dtypes.float8_e4m3

    # wetp[p, k, o] = (WSCALE*W_e^T)[k*128+p, o] (e4m3);
    # whtp[p, k*H + o] = W_h^T[k*128+p, o] (f16)
    wet_t = (W_attn[:, H:].T * np.float32(WSCALE)).astype(F8NP)  # (H, H)
    wht_t = W_attn[:, :H].T.astype(np.float16)
    wetp = np.ascontiguousarray(wet_t.reshape(KC, P, H).transpose(1, 0, 2))
    wet0 = np.ascontiguousarray(
        wet_t.reshape(KC, P, H)[:, :, :P].transpose(1, 0, 2)
    )
    whtp = np.ascontiguousarray(
        wht_t.reshape(KC, P, H).transpose(1, 0, 2).reshape(P, KC * H)
    )

    # packed constant block, f32 view (P, 128) / f16 view (P, 256)
    consts = np.zeros((P, 128), dtype=np.float32)
    c16 = consts.view(np.float16)  # (P, 256)
    consts[:, 0:16] = np.repeat(b_attn.reshape(OC, P).T, NB, axis=1)
    consts[:, 16:20] = v.reshape(OC, P).T
    for b in range(NB):
        for j in range(NB):
            consts[NB * b + j, 20 + b] = 1.0  # selb2 (rows 0:16)
            consts[b, 24 + NB * b + j] = 1.0  # selb (rows 0:4)
    c16[:, 80 + 15] = np.float16(1.0)  # onesw: ones at f16-col 15
    vrT = v.reshape(OC, P).T.astype(np.float16)  # (P, OC)
    for o in range(OC):
        c16[:, 128 + 32 * o + 15] = vrT[:, o]  # vwin sliding windows

    # (H, B, T) so per-core slices are cheap views before the copy
    enc_hbt = np.transpose(encoder_outputs, (2, 1, 0))
    in_maps = []
    for c in range(NCORES):
        b0 = c * NB
        # enct[p, k, r] = enc^T[k*128+p, r] (e4m3)
        enct = (
            np.ascontiguousarray(enc_hbt[:, b0 : b0 + NB, :])
            .astype(F8NP)
            .reshape(KC, P, ROWS)
        )
        enct = np.ascontiguousarray(enct.transpose(1, 0, 2))
        cc = consts.copy()
        cc16 = cc.view(np.float16)
        hidt = hidden[0, b0 : b0 + NB, :].T.astype(np.float16)  # (H, NB)
        for k in range(KC):
            cc16[:, 112 + NB * k : 112 + NB * (k + 1)] = hidt[
                k * P : (k + 1) * P, :
            ]
        in_maps.append(
            {
                "enct": enct,
                "wetp": wetp,
                "wet0": wet0,
                "whtp": whtp,
                "consts": cc,
            }
        )
    return in_maps


def kernel(hidden, encoder_outputs, W_attn, b_attn, v):
    nc = _get_nc()
    in_maps = _shard_inputs(hidden, encoder_outputs, W_attn, b_attn, v)
    res = bass_utils.run_bass_kernel_spmd(
        nc, in_maps, core_ids=list(range(NCORES))
    )
    outs = [res.results[c]["out"] for c in range(NCORES)]  # each (NB, T)
    full = np.concatenate(outs, axis=0)  # (B, T)
    return full[:, None, :].astype(np.float32)  # (B, 1, T)

